# revision 23
# baseline (speedup 1.0000x reference)
"""Trainium2 Bass kernel for MixLoRA sparse MoE (8 experts, top-2, shared base MLP).

Dense-mask formulation (no per-slot loops, no balancing): with top-2 routing,
  abar = wa*silu(common + d1) + wb*silu(common + d2),
  d_j  = B1cat_sc^T @ (mjx . u),   u = A1cat @ x  (all 8 experts stacked: 8*16=128 rows)
where m1x/m2x are host-shipped one-hot masks in (expert,rank)-row space and
wa = sigmoid(l_top1 - l_top2) (device-computed logits), wb = 1-wa. fc2 uses
  out = W2^T @ abar + B2cat_sc^T @ (m1x.z1 + m2x.z2) + b2,
  z_j = A2cat @ (w_j * a_j).
Sharding: 4 token quarters (T=512) x 2 H-halves (HL=2048); the host sums the
H-pair partials. All heavy matmuls are bf16 K=128 N=512; router fp32.
Perf structure: warmup matmuls engage the HAM clock-gate during the DMA head;
bf16 consts packed into one dram tensor loaded by priority-ordered DMAs
(gt+x first so the router can start); W1 prefetched in 4-tile chunks; W2
loaded mid-kernel. All DMAs on SyncE's hardware DGE (ScalarE's ring hangs
the device in this runtime). Outputs in bf16, host sums partials in fp32.
"""

import sys, os
sys.path.insert(0, "/opt/trn_rl_repo")

from contextlib import ExitStack

import numpy as np
import ml_dtypes

import concourse.bass as bass
import concourse.tile as tile
from concourse import mybir, bacc
from concourse.bass_utils import run_bass_kernel_spmd

BF = ml_dtypes.bfloat16

NCORES = 8
TQ = 4               # token shards
HH = 2               # H shards
D, H, E, R = 1024, 4096, 8, 16
NT = 2048
T = NT // TQ         # tokens per core (512)
HL = H // HH         # H per core (2048)
KD = D // 128        # 8
MH = HL // 128       # 16 local H slices
MD = D // 128        # 8
SC = 2.0
NWARM = 8

f32 = mybir.dt.float32
bf16 = mybir.dt.bfloat16

# column offsets in the packed bf16 consts tensor
C_GT = 0                     # [128, KD*E]  gate.T k-tiled       (64)
C_ONES = C_GT + KD * E       # [8, 128] ones (bf16, for l12 bcast) (128)
C_XTB = C_ONES + 128         # [128, KD*T]  x k-tiled            (4096)
C_A1P = C_XTB + KD * T       # [128, KD*128] A1cat.T k-tiled     (1024)
C_M1X = C_A1P + KD * 128     # [128, T]                          (512)
C_M2X = C_M1X + T            # [128, T]                          (512)
C_B1P = C_M2X + T            # [128, MH*128] B1cat_sc lhsT       (2048)
C_A2P = C_B1P + MH * 128     # [128, MH*128] A2cat.T lhsT        (2048)
C_B2P = C_A2P + MH * 128     # [128, MD*128] B2cat_sc lhsT       (1024)
C_END = C_B2P + MD * 128
G1A_END = C_XTB + 4 * T      # dma group 1a: gt + x k-tiles 0..3
G1B_END = C_M1X              # dma group 1b: x k-tiles 4..7 + a1p
G2_END = C_END               # dma group 2: masks, b1p, a2p, b2p

W1CH = 4                     # w1 m-tiles per dma chunk
W2CH = 4                     # w2 m2-tiles per dma chunk


def _build_bass(slots=0):
    nc = bacc.Bacc("TRN2", target_bir_lowering=False, debug=False)

    bigc = nc.dram_tensor("bigc", [128, C_END], bf16, kind="ExternalInput")
    sm8 = nc.dram_tensor("sm8", [8, T], f32, kind="ExternalInput")
    smc = nc.dram_tensor("smc", [128, MH + MD], f32, kind="ExternalInput")
    w1p = nc.dram_tensor("w1p", [MH // W1CH, 128, W1CH * KD * 128], bf16,
                         kind="ExternalInput")
    w2p = nc.dram_tensor("w2p", [MD // W2CH, 128, W2CH * MH * 128], bf16,
                         kind="ExternalInput")
    outt = nc.dram_tensor("outt", [128, MD * T], bf16, kind="ExternalOutput")

    with tile.TileContext(nc) as tc, ExitStack() as ctx:
        consts = ctx.enter_context(tc.tile_pool(name="consts", bufs=1))
        wpool = ctx.enter_context(tc.tile_pool(name="wpool", bufs=2))
        abufs = ctx.enter_context(tc.tile_pool(name="abufs", bufs=4))
        outp = ctx.enter_context(tc.tile_pool(name="outp", bufs=3))
        psF = ctx.enter_context(tc.tile_pool(name="psF", bufs=3, space="PSUM"))
        psZ = ctx.enter_context(tc.tile_pool(name="psZ", bufs=1, space="PSUM"))
        psR = ctx.enter_context(tc.tile_pool(name="psR", bufs=1, space="PSUM"))

        # ---- warmup: keep PE busy (and HAM warm) during the DMA head ----
        warm_w = consts.tile([128, 128], bf16, tag="warm_w")
        nc.vector.memset(warm_w, 0.0)
        warm_x = consts.tile([128, T], bf16, tag="warm_x")
        nc.vector.memset(warm_x, 0.0)
        warm_ps = psR.tile([128, T], f32, tag="wb", name="warm_ps")
        for i in range(NWARM):
            nc.tensor.matmul(warm_ps, warm_w, warm_x, start=True, stop=True)

        # ---- input DMAs, priority order ----
        bigc_sb = consts.tile([128, C_END], bf16, tag="bigc_sb")
        nc.sync.dma_start(bigc_sb[:, :G1A_END], bigc[:, :G1A_END])
        nc.sync.dma_start(bigc_sb[:, G1A_END:G1B_END], bigc[:, G1A_END:G1B_END])
        sm8_sb = consts.tile([8, T], f32, tag="sm8_sb")
        nc.sync.dma_start(sm8_sb, sm8[:])
        w1t = [None] * (MH // W1CH)
        w1t[0] = wpool.tile([128, W1CH * KD * 128], bf16, tag="w1t", name="w1t0")
        nc.sync.dma_start(w1t[0], w1p[0])
        nc.sync.dma_start(bigc_sb[:, C_M1X:C_B1P], bigc[:, C_M1X:C_B1P])
        smc_sb = consts.tile([128, MH + MD], f32, tag="smc_sb")
        nc.sync.dma_start(smc_sb, smc[:])
        nc.sync.dma_start(bigc_sb[:, C_B1P:C_B2P], bigc[:, C_B1P:C_B2P])
        w1t[1] = wpool.tile([128, W1CH * KD * 128], bf16, tag="w1t", name="w1t1")
        nc.sync.dma_start(w1t[1], w1p[1])
        nc.sync.dma_start(bigc_sb[:, C_B2P:C_END], bigc[:, C_B2P:C_END])

        def bc(c0, n):
            return bigc_sb[:, c0:c0 + n]

        def xtb_k(k):
            return bigc_sb[:, C_XTB + k * T:C_XTB + (k + 1) * T]

        dm8_sb = sm8_sb
        b1c_sb = smc_sb[:, :MH]
        b2c_sb = smc_sb[:, MH:]

        # ---- Router: lg[8,T] -> l12 broadcast to 128 partitions -> sigmoid ----
        lg_ps = psR.tile([8, T], f32, tag="lg", name="lg_ps")
        for k in range(KD):
            if k == 4:   # bridge the g1a->g1b DMA arrival gap with warmups
                for i in range(3):
                    nc.tensor.matmul(warm_ps, warm_w, warm_x, start=True, stop=True)
            nc.tensor.matmul(lg_ps, bc(C_GT + k * E, E), xtb_k(k),
                             start=(k == 0), stop=(k == KD - 1))
        mlg_sb = consts.tile([8, T], bf16, tag="mlg_sb")
        nc.vector.tensor_tensor(mlg_sb, lg_ps, dm8_sb, op=mybir.AluOpType.mult)
        wb_ps = psR.tile([128, T], f32, tag="wb", name="wb_ps")
        nc.tensor.matmul(wb_ps, bigc_sb[0:8, C_ONES:C_ONES + 128], mlg_sb,
                         start=True, stop=True)
        wab_sb = consts.tile([128, T], bf16, tag="wab_sb")
        nc.scalar.activation(wab_sb, wb_ps, mybir.ActivationFunctionType.Sigmoid)
        wbb_sb = consts.tile([128, T], bf16, tag="wbb_sb")
        nc.vector.tensor_scalar(wbb_sb, wab_sb, -1.0, 1.0,
                                op0=mybir.AluOpType.mult,
                                op1=mybir.AluOpType.add)

        # ---- u = A1cat @ x; masked cu1/cu2; cud = cu2-cu1 ----
        u_ps = psR.tile([128, T], f32, tag="u", name="u_ps")
        for k in range(KD):
            nc.tensor.matmul(u_ps, bc(C_A1P + k * 128, 128), xtb_k(k),
                             start=(k == 0), stop=(k == KD - 1))
        cu1_sb = consts.tile([128, T], bf16, tag="cu1_sb")
        nc.vector.tensor_tensor(cu1_sb, u_ps, bc(C_M1X, T), op=mybir.AluOpType.mult)
        cu2_sb = consts.tile([128, T], bf16, tag="cu2_sb")
        nc.vector.tensor_tensor(cu2_sb, u_ps, bc(C_M2X, T), op=mybir.AluOpType.mult)
        cud_sb = consts.tile([128, T], bf16, tag="cud_sb")
        nc.vector.tensor_tensor(cud_sb, cu2_sb, cu1_sb, op=mybir.AluOpType.subtract)

        # ---- fc1 m-loop: common+d1 -> silu -> aw1; +(d2-d1) -> silu -> aw2 ----
        aw1_all = consts.tile([128, MH * T], bf16, tag="aw1_all")
        aw2_all = consts.tile([128, MH * T], bf16, tag="aw2_all")
        abar_all = consts.tile([128, MH * T], bf16, tag="abar_all")
        w2_sb = consts.tile([128, MD * MH * 128], bf16, tag="w2_sb")
        z1_ps = psZ.tile([128, T], f32, tag="z1", name="z1_ps")
        z2_ps = psZ.tile([128, T], f32, tag="z2", name="z2_ps")
        for m in range(MH):
            msl = slice(m * T, (m + 1) * T)
            ch, mi = divmod(m, W1CH)
            if mi == 2 and ch + 2 < MH // W1CH:  # prefetch chunk ch+2 (ring of 2)
                w1t[ch + 2] = wpool.tile([128, W1CH * KD * 128], bf16, tag="w1t",
                                         name=f"w1t{ch + 2}")
                nc.sync.dma_start(w1t[ch + 2], w1p[ch + 2])
            if m == 4:               # mid-kernel W2 loads
                nc.sync.dma_start(w2_sb[:, :MD * MH * 64], w2p[0])
            if m == 8:
                nc.sync.dma_start(w2_sb[:, MD * MH * 64:], w2p[1])
            w1m = w1t[ch]
            f_ps = psF.tile([128, T], f32, tag="mm", name="f_ps")
            for k in range(KD):
                nc.tensor.matmul(f_ps, w1m[:, (mi * KD + k) * 128:(mi * KD + k + 1) * 128],
                                 xtb_k(k), start=(k == 0), stop=False)
            nc.tensor.matmul(f_ps, bc(C_B1P + m * 128, 128), cu1_sb,
                             start=False, stop=True)
            a1t = abufs.tile([128, T], bf16, tag="a1t", name="a1t")
            nc.scalar.activation(a1t, f_ps, mybir.ActivationFunctionType.Silu,
                                 bias=b1c_sb[:, m:m + 1])
            nc.vector.tensor_tensor(aw1_all[:, msl], a1t, wab_sb,
                                    op=mybir.AluOpType.mult)
            nc.tensor.matmul(f_ps, bc(C_B1P + m * 128, 128), cud_sb,
                             start=False, stop=True, skip_group_check=True)
            a2t = abufs.tile([128, T], bf16, tag="a2t", name="a2t")
            nc.scalar.activation(a2t, f_ps, mybir.ActivationFunctionType.Silu,
                                 bias=b1c_sb[:, m:m + 1])
            nc.vector.tensor_tensor(aw2_all[:, msl], a2t, wbb_sb,
                                    op=mybir.AluOpType.mult)
            nc.tensor.matmul(z1_ps, bc(C_A2P + m * 128, 128),
                             aw1_all[:, msl], start=(m == 0), stop=(m == MH - 1))
            nc.tensor.matmul(z2_ps, bc(C_A2P + m * 128, 128),
                             aw2_all[:, msl], start=(m == 0), stop=(m == MH - 1))
            nc.vector.tensor_tensor(abar_all[:, msl], aw1_all[:, msl],
                                    aw2_all[:, msl], op=mybir.AluOpType.add)

        # ---- v2 = m1x.z1 + m2x.z2 ----
        zt1 = consts.tile([128, T], bf16, tag="zt1")
        nc.vector.tensor_tensor(zt1, z1_ps, bc(C_M1X, T), op=mybir.AluOpType.mult)
        zt2 = consts.tile([128, T], bf16, tag="zt2")
        nc.vector.tensor_tensor(zt2, z2_ps, bc(C_M2X, T), op=mybir.AluOpType.mult)
        v2_sb = consts.tile([128, T], bf16, tag="v2_sb")
        nc.vector.tensor_tensor(v2_sb, zt1, zt2, op=mybir.AluOpType.add)

        # ---- fc2: W2half^T @ abar + B2cat_sc^T @ v2 (+ b2 on hh==0) ----
        # Last tile runs as two half-width groups (in the router's dead PSUM
        # banks) so half its copy+DMA epilogue hides under the other half's MMs.
        for m2 in range(MD):
            halves = [(0, T)] if m2 < MD - 1 else [(0, T // 2), (T // 2, T)]
            for hi, (h0, h1) in enumerate(halves):
                hn = h1 - h0
                if m2 < MD - 1:
                    o_ps = psF.tile([128, T], f32, tag="mm")
                else:
                    o_psf = psR.tile([128, T], f32, tag=("u" if hi == 0 else "wb"),
                                     name=f"oh{hi}")
                    o_ps = o_psf[:, :hn]
                for k2 in range(MH):
                    nc.tensor.matmul(
                        o_ps, w2_sb[:, (m2 * MH + k2) * 128:(m2 * MH + k2 + 1) * 128],
                        abar_all[:, k2 * T + h0:k2 * T + h1],
                        start=(k2 == 0), stop=False)
                nc.tensor.matmul(o_ps, bc(C_B2P + m2 * 128, 128), v2_sb[:, h0:h1],
                                 start=False, stop=True)
                o_sbf = outp.tile([128, T], bf16, tag="osb", name=f"osb{m2}_{hi}")
                o_sb = o_sbf[:, :hn]
                nc.vector.tensor_scalar(o_sb, o_ps, b2c_sb[:, m2:m2 + 1], None,
                                        op0=mybir.AluOpType.add)
                nc.sync.dma_start(outt[:, m2 * T + h0:m2 * T + h1], o_sb)

    nc.compile()
    return nc


def _pack_inputs(hidden_states, gate, W1, b1, W2, b2, A1, B1, A2, B2):
    hs = np.asarray(hidden_states, dtype=np.float32)
    x = hs.reshape(NT, D)
    gate = np.asarray(gate, np.float32)

    # Host routing: top-2 selection masks only (weights computed on device).
    logits = x @ gate.T
    order = np.argsort(-logits, axis=1, kind="stable")
    m1 = np.zeros((NT, E), np.float32)
    m2 = np.zeros((NT, E), np.float32)
    np.put_along_axis(m1, order[:, :1], 1.0, axis=1)
    np.put_along_axis(m2, order[:, 1:2], 1.0, axis=1)
    m1xf = np.repeat(m1, R, axis=1)          # [NT, 128]
    m2xf = np.repeat(m2, R, axis=1)
    dm8f = (m1 - m2).T                       # [E, NT]

    xT = np.ascontiguousarray(x.T)           # [D, NT]

    gT = gate.T
    gt = np.ascontiguousarray(
        gT.reshape(KD, 128, E).transpose(1, 0, 2).reshape(128, KD * E)).astype(BF)

    W1T = np.asarray(W1, np.float32).T       # [D, H]
    w1p_full = np.ascontiguousarray(
        W1T.reshape(KD, 128, H // 128, 128).transpose(2, 1, 0, 3)
        .reshape(H // 128, 128, KD * 128)).astype(BF)
    W2T = np.asarray(W2, np.float32).T       # [H, D]
    w2p_full = np.ascontiguousarray(
        W2T.reshape(H // 128, 128, MD, 128).transpose(2, 1, 0, 3)
        .reshape(MD, 128, (H // 128) * 128)).astype(BF)

    A1 = np.asarray(A1, np.float32)
    B1 = np.asarray(B1, np.float32)
    A2 = np.asarray(A2, np.float32)
    B2 = np.asarray(B2, np.float32)

    A1cat = A1.reshape(E * R, D)                                    # [128, D]
    a1p = np.ascontiguousarray(
        A1cat.T.reshape(KD, 128, 128).transpose(1, 0, 2)
        .reshape(128, KD * 128)).astype(BF)
    B1cat = SC * np.concatenate([B1[e] for e in range(E)], axis=1)  # [H, 128]
    A2cat = A2.reshape(E * R, H)                                    # [128, H]
    B2cat = SC * np.concatenate([B2[e] for e in range(E)], axis=1)  # [D, 128]
    # b2p[m2]: lhsT = B2cat[m2-tile rows].T -> [128(er), 128(D-cols)]
    b2p = np.ascontiguousarray(
        B2cat.reshape(MD, 128, 128).transpose(2, 0, 1)
        .reshape(128, MD * 128)).astype(BF)

    b1c_full = np.ascontiguousarray(
        np.asarray(b1, np.float32).reshape(H // 128, 128).T)        # [128, 32]
    b2c = np.ascontiguousarray(np.asarray(b2, np.float32).reshape(MD, 128).T)
    b2c_zero = np.zeros_like(b2c)

    in_maps = []
    for c in range(NCORES):
        tq, hh = divmod(c, HH)
        tsl = slice(tq * T, (tq + 1) * T)
        xc = xT[:, tsl]
        xcp = np.ascontiguousarray(
            xc.reshape(KD, 128, T).transpose(1, 0, 2).reshape(128, KD * T))
        hsl = slice(hh * HL, (hh + 1) * HL)
        msl = slice(hh * MH, (hh + 1) * MH)
        # b1p[m]: lhsT = B1cat[hh-local m-tile rows].T -> [128(er), 128(H-cols)]
        b1ph = np.ascontiguousarray(
            B1cat[hsl].reshape(MH, 128, 128).transpose(2, 0, 1)
            .reshape(128, MH * 128)).astype(BF)
        # a2p[m]: lhsT = A2cat[:, hh-local m-tile].T -> [128(H-rows), 128(er)]
        a2ph = np.ascontiguousarray(
            A2cat[:, hsl].T.reshape(MH, 128, 128).transpose(1, 0, 2)
            .reshape(128, MH * 128)).astype(BF)
        ones_blk = np.zeros((128, 128), np.float32)
        ones_blk[:8] = 1.0
        bigc_np = np.concatenate([
            gt,
            ones_blk.astype(BF),
            xcp.astype(BF),
            a1p,
            np.ascontiguousarray(m1xf[tsl].T).astype(BF),
            np.ascontiguousarray(m2xf[tsl].T).astype(BF),
            b1ph,
            a2ph,
            b2p,
        ], axis=1)
        sm8 = np.ascontiguousarray(dm8f[:, tsl])
        smc = np.concatenate([
            np.ascontiguousarray(b1c_full[:, msl]),
            b2c if hh == 0 else b2c_zero,
        ], axis=1)
        w1c = np.ascontiguousarray(
            w1p_full[msl].reshape(MH // W1CH, W1CH, 128, KD * 128)
            .transpose(0, 2, 1, 3).reshape(MH // W1CH, 128, W1CH * KD * 128))
        # w2 chunk c covers m2 in [c*W2CH, (c+1)*W2CH), flattened (m2, k2)-major
        w2h = w2p_full[:, :, hh * MH * 128:(hh + 1) * MH * 128]     # [MD,128,MH*128]
        w2c = np.ascontiguousarray(
            w2h.reshape(MD // W2CH, W2CH, 128, MH * 128)
            .transpose(0, 2, 1, 3).reshape(MD // W2CH, 128, W2CH * MH * 128))
        in_maps.append({
            "bigc": bigc_np,
            "sm8": sm8,
            "smc": smc,
            "w1p": w1c,
            "w2p": w2c,
        })
    return in_maps, np.arange(NT), 0


_NC_CACHE = {}


def get_nc(slots=0):
    if slots not in _NC_CACHE:
        _NC_CACHE[slots] = _build_bass(slots)
    return _NC_CACHE[slots]


def _unpack_outputs(results, perm):
    cols = []
    for tq in range(TQ):
        o = None
        for hh in range(HH):
            c = tq * HH + hh
            p = np.asarray(results[c]["outt"], np.float32)
            p = p.reshape(128, MD, T).transpose(1, 0, 2).reshape(D, T)
            o = p if o is None else o + p
        cols.append(o)
    outT = np.concatenate(cols, axis=1)                  # [D, NT]
    out = np.empty((NT, D), np.float32)
    out[perm] = outT.T
    return out.reshape(2, NT // 2, D)


def kernel(**inputs):
    in_maps, perm, slots = _pack_inputs(**inputs)
    nc = get_nc(slots)
    res = run_bass_kernel_spmd(nc, in_maps, core_ids=list(range(NCORES)))
    return _unpack_outputs(res.results, perm)


# revision 25
# speedup vs baseline: 1.0280x; 1.0280x over previous
"""Trainium2 Bass kernel for MixLoRA sparse MoE (8 experts, top-2, shared base MLP).

Dense-mask formulation (no per-slot loops, no balancing): with top-2 routing,
  abar = wa*silu(common + d1) + wb*silu(common + d2),
  d_j  = B1cat_sc^T @ (mjx . u),   u = A1cat @ x  (all 8 experts stacked: 8*16=128 rows)
where m1x/m2x are host-shipped one-hot masks in (expert,rank)-row space and
wa = sigmoid(l_top1 - l_top2) (device-computed logits), wb = 1-wa. fc2 uses
  out = W2^T @ abar + B2cat_sc^T @ (m1x.z1 + m2x.z2) + b2,
  z_j = A2cat @ (w_j * a_j).
Sharding: 4 token quarters (T=512) x 2 H-halves (HL=2048); the host sums the
H-pair partials. All heavy matmuls are bf16 K=128 N=512; router fp32.
Perf structure: warmup matmuls engage the HAM clock-gate during the DMA head;
bf16 consts packed into one dram tensor loaded by priority-ordered DMAs
(gt+x first so the router can start); W1 prefetched in 4-tile chunks; W2
loaded mid-kernel. All DMAs on SyncE's hardware DGE (ScalarE's ring hangs
the device in this runtime). Outputs in bf16, host sums partials in fp32.
"""

import sys, os
sys.path.insert(0, "/opt/trn_rl_repo")

from contextlib import ExitStack

import numpy as np
import ml_dtypes

import concourse.bass as bass
import concourse.tile as tile
from concourse import mybir, bacc
from concourse.bass_utils import run_bass_kernel_spmd

BF = ml_dtypes.bfloat16

NCORES = 8
TQ = 4               # token shards
HH = 2               # H shards
D, H, E, R = 1024, 4096, 8, 16
NT = 2048
T = NT // TQ         # tokens per core (512)
HL = H // HH         # H per core (2048)
KD = D // 128        # 8
MH = HL // 128       # 16 local H slices
MD = D // 128        # 8
SC = 2.0
NWARM = 8

f32 = mybir.dt.float32
bf16 = mybir.dt.bfloat16

# column offsets in the packed bf16 consts tensor
C_GT = 0                     # [128, KD*E]  gate.T k-tiled       (64)
C_ONES = C_GT + KD * E       # [8, 128] ones (bf16, for l12 bcast) (128)
C_XTB = C_ONES + 128         # [128, KD*T]  x k-tiled            (4096)
C_A1P = C_XTB + KD * T       # [128, KD*128] A1cat.T k-tiled     (1024)
C_M1X = C_A1P + KD * 128     # [128, T]                          (512)
C_M2X = C_M1X + T            # [128, T]                          (512)
C_B1P = C_M2X + T            # [128, MH*128] B1cat_sc lhsT       (2048)
C_A2P = C_B1P + MH * 128     # [128, MH*128] A2cat.T lhsT        (2048)
C_B2P = C_A2P + MH * 128     # [128, MD*128] B2cat_sc lhsT       (1024)
C_END = C_B2P + MD * 128
G1A_END = C_XTB + 4 * T      # dma group 1a: gt + x k-tiles 0..3
G1B_END = C_M1X              # dma group 1b: x k-tiles 4..7 + a1p
G2_END = C_END               # dma group 2: masks, b1p, a2p, b2p

W1CH = 4                     # w1 m-tiles per dma chunk
W2CH = 4                     # w2 m2-tiles per dma chunk


def _build_bass(slots=0):
    nc = bacc.Bacc("TRN2", target_bir_lowering=False, debug=False)

    bigc = nc.dram_tensor("bigc", [128, C_END], bf16, kind="ExternalInput")
    sm8 = nc.dram_tensor("sm8", [8, T], f32, kind="ExternalInput")
    smc = nc.dram_tensor("smc", [128, MH + MD], f32, kind="ExternalInput")
    w1p = nc.dram_tensor("w1p", [MH // W1CH, 128, W1CH * KD * 128], bf16,
                         kind="ExternalInput")
    w2p = nc.dram_tensor("w2p", [MD // W2CH, 128, W2CH * MH * 128], bf16,
                         kind="ExternalInput")
    outt = nc.dram_tensor("outt", [128, MD * T], bf16, kind="ExternalOutput")

    with tile.TileContext(nc) as tc, ExitStack() as ctx:
        consts = ctx.enter_context(tc.tile_pool(name="consts", bufs=1))
        wpool = ctx.enter_context(tc.tile_pool(name="wpool", bufs=2))
        abufs = ctx.enter_context(tc.tile_pool(name="abufs", bufs=4))
        outp = ctx.enter_context(tc.tile_pool(name="outp", bufs=3))
        psF = ctx.enter_context(tc.tile_pool(name="psF", bufs=3, space="PSUM"))
        psZ = ctx.enter_context(tc.tile_pool(name="psZ", bufs=1, space="PSUM"))
        psR = ctx.enter_context(tc.tile_pool(name="psR", bufs=1, space="PSUM"))

        # ---- warmup: keep PE busy (and HAM warm) during the DMA head ----
        warm_w = consts.tile([128, 128], bf16, tag="warm_w")
        nc.vector.memset(warm_w, 0.0)
        warm_x = consts.tile([128, T], bf16, tag="warm_x")
        nc.vector.memset(warm_x, 0.0)
        warm_ps = psR.tile([128, T], f32, tag="wb", name="warm_ps")
        for i in range(NWARM):
            nc.tensor.matmul(warm_ps, warm_w, warm_x, start=True, stop=True)

        # ---- input DMAs, priority order ----
        bigc_sb = consts.tile([128, C_END], bf16, tag="bigc_sb")
        nc.sync.dma_start(bigc_sb[:, :G1A_END], bigc[:, :G1A_END])
        nc.sync.dma_start(bigc_sb[:, G1A_END:G1B_END], bigc[:, G1A_END:G1B_END])
        sm8_sb = consts.tile([8, T], f32, tag="sm8_sb")
        nc.sync.dma_start(sm8_sb, sm8[:])
        w1t = [None] * (MH // W1CH)
        w1t[0] = wpool.tile([128, W1CH * KD * 128], bf16, tag="w1t", name="w1t0")
        nc.sync.dma_start(w1t[0], w1p[0])
        nc.sync.dma_start(bigc_sb[:, C_M1X:C_B1P], bigc[:, C_M1X:C_B1P])
        smc_sb = consts.tile([128, MH + MD], f32, tag="smc_sb")
        nc.sync.dma_start(smc_sb, smc[:])
        nc.sync.dma_start(bigc_sb[:, C_B1P:C_B2P], bigc[:, C_B1P:C_B2P])
        w1t[1] = wpool.tile([128, W1CH * KD * 128], bf16, tag="w1t", name="w1t1")
        nc.sync.dma_start(w1t[1], w1p[1])
        nc.sync.dma_start(bigc_sb[:, C_B2P:C_END], bigc[:, C_B2P:C_END])

        def bc(c0, n):
            return bigc_sb[:, c0:c0 + n]

        def xtb_k(k):
            return bigc_sb[:, C_XTB + k * T:C_XTB + (k + 1) * T]

        dm8_sb = sm8_sb
        b1c_sb = smc_sb[:, :MH]
        b2c_sb = smc_sb[:, MH:]

        # ---- Router: lg[8,T] -> l12 broadcast to 128 partitions -> sigmoid ----
        lg_ps = psR.tile([8, T], f32, tag="lg", name="lg_ps")
        for k in range(KD):
            nc.tensor.matmul(lg_ps, bc(C_GT + k * E, E), xtb_k(k),
                             start=(k == 0), stop=(k == KD - 1))
        mlg_sb = consts.tile([8, T], bf16, tag="mlg_sb")
        nc.vector.tensor_tensor(mlg_sb, lg_ps, dm8_sb, op=mybir.AluOpType.mult)
        wb_ps = psR.tile([128, T], f32, tag="wb", name="wb_ps")
        nc.tensor.matmul(wb_ps, bigc_sb[0:8, C_ONES:C_ONES + 128], mlg_sb,
                         start=True, stop=True)
        wab_sb = consts.tile([128, T], bf16, tag="wab_sb")
        nc.scalar.activation(wab_sb, wb_ps, mybir.ActivationFunctionType.Sigmoid)
        wbb_sb = consts.tile([128, T], bf16, tag="wbb_sb")
        nc.vector.tensor_scalar(wbb_sb, wab_sb, -1.0, 1.0,
                                op0=mybir.AluOpType.mult,
                                op1=mybir.AluOpType.add)

        # ---- u = A1cat @ x; masked cu1/cu2; cud = cu2-cu1 ----
        u_ps = psR.tile([128, T], f32, tag="u", name="u_ps")
        for k in range(KD):
            nc.tensor.matmul(u_ps, bc(C_A1P + k * 128, 128), xtb_k(k),
                             start=(k == 0), stop=(k == KD - 1))
        cu1_sb = consts.tile([128, T], bf16, tag="cu1_sb")
        nc.vector.tensor_tensor(cu1_sb, u_ps, bc(C_M1X, T), op=mybir.AluOpType.mult)
        cu2_sb = consts.tile([128, T], bf16, tag="cu2_sb")
        nc.vector.tensor_tensor(cu2_sb, u_ps, bc(C_M2X, T), op=mybir.AluOpType.mult)
        cud_sb = consts.tile([128, T], bf16, tag="cud_sb")
        nc.vector.tensor_tensor(cud_sb, cu2_sb, cu1_sb, op=mybir.AluOpType.subtract)

        # ---- fc1 m-loop: common+d1 -> silu -> aw1; +(d2-d1) -> silu -> aw2 ----
        aw1_all = consts.tile([128, MH * T], bf16, tag="aw1_all")
        aw2_all = consts.tile([128, MH * T], bf16, tag="aw2_all")
        abar_all = consts.tile([128, MH * T], bf16, tag="abar_all")
        w2_sb = consts.tile([128, MD * MH * 128], bf16, tag="w2_sb")
        z1_ps = psZ.tile([128, T], f32, tag="z1", name="z1_ps")
        z2_ps = psZ.tile([128, T], f32, tag="z2", name="z2_ps")
        # Software-pipelined emission: tile m's dependent matmuls (d2, z1, z2)
        # are emitted 1-2 tiles behind its fills so the PE FIFO always has a
        # full tile of independent work between a producer (ACT/DVE) and its
        # consumer matmul — their semaphore waits then never stall the PE.
        fps = {}
        for m in range(MH + 2):
            if m < MH:
                msl = slice(m * T, (m + 1) * T)
                ch, mi = divmod(m, W1CH)
                if mi == 2 and ch + 2 < MH // W1CH:  # prefetch chunk ch+2
                    w1t[ch + 2] = wpool.tile([128, W1CH * KD * 128], bf16,
                                             tag="w1t", name=f"w1t{ch + 2}")
                    nc.sync.dma_start(w1t[ch + 2], w1p[ch + 2])
                if m == 4:               # mid-kernel W2 loads
                    nc.sync.dma_start(w2_sb[:, :MD * MH * 64], w2p[0])
                if m == 8:
                    nc.sync.dma_start(w2_sb[:, MD * MH * 64:], w2p[1])
                w1m = w1t[ch]
                f_ps = psF.tile([128, T], f32, tag="mm", name="f_ps")
                fps[m] = f_ps
                for k in range(KD):
                    nc.tensor.matmul(
                        f_ps, w1m[:, (mi * KD + k) * 128:(mi * KD + k + 1) * 128],
                        xtb_k(k), start=(k == 0), stop=False)
                nc.tensor.matmul(f_ps, bc(C_B1P + m * 128, 128), cu1_sb,
                                 start=False, stop=True)
                a1t = abufs.tile([128, T], bf16, tag="a1t", name="a1t")
                nc.scalar.activation(a1t, f_ps, mybir.ActivationFunctionType.Silu,
                                     bias=b1c_sb[:, m:m + 1])
                nc.vector.tensor_tensor(aw1_all[:, msl], a1t, wab_sb,
                                        op=mybir.AluOpType.mult)
            if 1 <= m and m - 1 < MH:
                m1 = m - 1
                msl1 = slice(m1 * T, (m1 + 1) * T)
                nc.tensor.matmul(fps[m1], bc(C_B1P + m1 * 128, 128), cud_sb,
                                 start=False, stop=True, skip_group_check=True)
                a2t = abufs.tile([128, T], bf16, tag="a2t", name="a2t")
                nc.scalar.activation(a2t, fps[m1], mybir.ActivationFunctionType.Silu,
                                     bias=b1c_sb[:, m1:m1 + 1])
                nc.vector.tensor_tensor(aw2_all[:, msl1], a2t, wbb_sb,
                                        op=mybir.AluOpType.mult)
                nc.tensor.matmul(z1_ps, bc(C_A2P + m1 * 128, 128),
                                 aw1_all[:, msl1], start=(m1 == 0),
                                 stop=(m1 == MH - 1))
            if 2 <= m and m - 2 < MH:
                m2i = m - 2
                msl2 = slice(m2i * T, (m2i + 1) * T)
                nc.tensor.matmul(z2_ps, bc(C_A2P + m2i * 128, 128),
                                 aw2_all[:, msl2], start=(m2i == 0),
                                 stop=(m2i == MH - 1))
                nc.vector.tensor_tensor(abar_all[:, msl2], aw1_all[:, msl2],
                                        aw2_all[:, msl2], op=mybir.AluOpType.add)
                del fps[m2i]

        # ---- v2 = m1x.z1 + m2x.z2 ----
        zt1 = consts.tile([128, T], bf16, tag="zt1")
        nc.vector.tensor_tensor(zt1, z1_ps, bc(C_M1X, T), op=mybir.AluOpType.mult)
        zt2 = consts.tile([128, T], bf16, tag="zt2")
        nc.vector.tensor_tensor(zt2, z2_ps, bc(C_M2X, T), op=mybir.AluOpType.mult)
        v2_sb = consts.tile([128, T], bf16, tag="v2_sb")
        nc.vector.tensor_tensor(v2_sb, zt1, zt2, op=mybir.AluOpType.add)

        # ---- fc2: W2half^T @ abar + B2cat_sc^T @ v2 (+ b2 on hh==0) ----
        # Last tile runs as two half-width groups (in the router's dead PSUM
        # banks) so half its copy+DMA epilogue hides under the other half's MMs.
        for m2 in range(MD):
            halves = [(0, T)] if m2 < MD - 1 else [(0, T // 2), (T // 2, T)]
            for hi, (h0, h1) in enumerate(halves):
                hn = h1 - h0
                if m2 < MD - 1:
                    o_ps = psF.tile([128, T], f32, tag="mm")
                else:
                    o_psf = psR.tile([128, T], f32, tag=("u" if hi == 0 else "wb"),
                                     name=f"oh{hi}")
                    o_ps = o_psf[:, :hn]
                for k2 in range(MH):
                    nc.tensor.matmul(
                        o_ps, w2_sb[:, (m2 * MH + k2) * 128:(m2 * MH + k2 + 1) * 128],
                        abar_all[:, k2 * T + h0:k2 * T + h1],
                        start=(k2 == 0), stop=False)
                nc.tensor.matmul(o_ps, bc(C_B2P + m2 * 128, 128), v2_sb[:, h0:h1],
                                 start=False, stop=True)
                o_sbf = outp.tile([128, T], bf16, tag="osb", name=f"osb{m2}_{hi}")
                o_sb = o_sbf[:, :hn]
                nc.vector.tensor_scalar(o_sb, o_ps, b2c_sb[:, m2:m2 + 1], None,
                                        op0=mybir.AluOpType.add)
                nc.sync.dma_start(outt[:, m2 * T + h0:m2 * T + h1], o_sb)

    nc.compile()
    return nc


def _pack_inputs(hidden_states, gate, W1, b1, W2, b2, A1, B1, A2, B2):
    hs = np.asarray(hidden_states, dtype=np.float32)
    x = hs.reshape(NT, D)
    gate = np.asarray(gate, np.float32)

    # Host routing: top-2 selection masks only (weights computed on device).
    logits = x @ gate.T
    order = np.argsort(-logits, axis=1, kind="stable")
    m1 = np.zeros((NT, E), np.float32)
    m2 = np.zeros((NT, E), np.float32)
    np.put_along_axis(m1, order[:, :1], 1.0, axis=1)
    np.put_along_axis(m2, order[:, 1:2], 1.0, axis=1)
    m1xf = np.repeat(m1, R, axis=1)          # [NT, 128]
    m2xf = np.repeat(m2, R, axis=1)
    dm8f = (m1 - m2).T                       # [E, NT]

    xT = np.ascontiguousarray(x.T)           # [D, NT]

    gT = gate.T
    gt = np.ascontiguousarray(
        gT.reshape(KD, 128, E).transpose(1, 0, 2).reshape(128, KD * E)).astype(BF)

    W1T = np.asarray(W1, np.float32).T       # [D, H]
    w1p_full = np.ascontiguousarray(
        W1T.reshape(KD, 128, H // 128, 128).transpose(2, 1, 0, 3)
        .reshape(H // 128, 128, KD * 128)).astype(BF)
    W2T = np.asarray(W2, np.float32).T       # [H, D]
    w2p_full = np.ascontiguousarray(
        W2T.reshape(H // 128, 128, MD, 128).transpose(2, 1, 0, 3)
        .reshape(MD, 128, (H // 128) * 128)).astype(BF)

    A1 = np.asarray(A1, np.float32)
    B1 = np.asarray(B1, np.float32)
    A2 = np.asarray(A2, np.float32)
    B2 = np.asarray(B2, np.float32)

    A1cat = A1.reshape(E * R, D)                                    # [128, D]
    a1p = np.ascontiguousarray(
        A1cat.T.reshape(KD, 128, 128).transpose(1, 0, 2)
        .reshape(128, KD * 128)).astype(BF)
    B1cat = SC * np.concatenate([B1[e] for e in range(E)], axis=1)  # [H, 128]
    A2cat = A2.reshape(E * R, H)                                    # [128, H]
    B2cat = SC * np.concatenate([B2[e] for e in range(E)], axis=1)  # [D, 128]
    # b2p[m2]: lhsT = B2cat[m2-tile rows].T -> [128(er), 128(D-cols)]
    b2p = np.ascontiguousarray(
        B2cat.reshape(MD, 128, 128).transpose(2, 0, 1)
        .reshape(128, MD * 128)).astype(BF)

    b1c_full = np.ascontiguousarray(
        np.asarray(b1, np.float32).reshape(H // 128, 128).T)        # [128, 32]
    b2c = np.ascontiguousarray(np.asarray(b2, np.float32).reshape(MD, 128).T)
    b2c_zero = np.zeros_like(b2c)

    in_maps = []
    for c in range(NCORES):
        tq, hh = divmod(c, HH)
        tsl = slice(tq * T, (tq + 1) * T)
        xc = xT[:, tsl]
        xcp = np.ascontiguousarray(
            xc.reshape(KD, 128, T).transpose(1, 0, 2).reshape(128, KD * T))
        hsl = slice(hh * HL, (hh + 1) * HL)
        msl = slice(hh * MH, (hh + 1) * MH)
        # b1p[m]: lhsT = B1cat[hh-local m-tile rows].T -> [128(er), 128(H-cols)]
        b1ph = np.ascontiguousarray(
            B1cat[hsl].reshape(MH, 128, 128).transpose(2, 0, 1)
            .reshape(128, MH * 128)).astype(BF)
        # a2p[m]: lhsT = A2cat[:, hh-local m-tile].T -> [128(H-rows), 128(er)]
        a2ph = np.ascontiguousarray(
            A2cat[:, hsl].T.reshape(MH, 128, 128).transpose(1, 0, 2)
            .reshape(128, MH * 128)).astype(BF)
        ones_blk = np.zeros((128, 128), np.float32)
        ones_blk[:8] = 1.0
        bigc_np = np.concatenate([
            gt,
            ones_blk.astype(BF),
            xcp.astype(BF),
            a1p,
            np.ascontiguousarray(m1xf[tsl].T).astype(BF),
            np.ascontiguousarray(m2xf[tsl].T).astype(BF),
            b1ph,
            a2ph,
            b2p,
        ], axis=1)
        sm8 = np.ascontiguousarray(dm8f[:, tsl])
        smc = np.concatenate([
            np.ascontiguousarray(b1c_full[:, msl]),
            b2c if hh == 0 else b2c_zero,
        ], axis=1)
        w1c = np.ascontiguousarray(
            w1p_full[msl].reshape(MH // W1CH, W1CH, 128, KD * 128)
            .transpose(0, 2, 1, 3).reshape(MH // W1CH, 128, W1CH * KD * 128))
        # w2 chunk c covers m2 in [c*W2CH, (c+1)*W2CH), flattened (m2, k2)-major
        w2h = w2p_full[:, :, hh * MH * 128:(hh + 1) * MH * 128]     # [MD,128,MH*128]
        w2c = np.ascontiguousarray(
            w2h.reshape(MD // W2CH, W2CH, 128, MH * 128)
            .transpose(0, 2, 1, 3).reshape(MD // W2CH, 128, W2CH * MH * 128))
        in_maps.append({
            "bigc": bigc_np,
            "sm8": sm8,
            "smc": smc,
            "w1p": w1c,
            "w2p": w2c,
        })
    return in_maps, np.arange(NT), 0


_NC_CACHE = {}


def get_nc(slots=0):
    if slots not in _NC_CACHE:
        _NC_CACHE[slots] = _build_bass(slots)
    return _NC_CACHE[slots]


def _unpack_outputs(results, perm):
    cols = []
    for tq in range(TQ):
        o = None
        for hh in range(HH):
            c = tq * HH + hh
            p = np.asarray(results[c]["outt"], np.float32)
            p = p.reshape(128, MD, T).transpose(1, 0, 2).reshape(D, T)
            o = p if o is None else o + p
        cols.append(o)
    outT = np.concatenate(cols, axis=1)                  # [D, NT]
    out = np.empty((NT, D), np.float32)
    out[perm] = outT.T
    return out.reshape(2, NT // 2, D)


def kernel(**inputs):
    in_maps, perm, slots = _pack_inputs(**inputs)
    nc = get_nc(slots)
    res = run_bass_kernel_spmd(nc, in_maps, core_ids=list(range(NCORES)))
    return _unpack_outputs(res.results, perm)
